# revision 24
# baseline (speedup 1.0000x reference)
"""Trainium2 Bass kernel for nn_BrainInspiredRouter.

Math (reference, seq_len==1 attention => attn collapses to the V path):
    attended = x @ (out_proj_w @ Wv).T + (out_proj_w @ bv + out_proj_b)
    h        = relu(attended @ W1[r].T + b1[r])          per route r
    route    = h @ W2[r].T + b2[r]
    gate     = softmax(x @ Wg.T + bg)
    out      = sum_r gate[:, r] * route[:, r, :]

Host-side constant folding (weights only, no activations):
    W1f[r]  = W1[r] @ (out_proj_w @ Wv)      -> h = relu(x @ W1f.T + b1f)
    b1f[r]  = W1[r] @ (out_proj_w@bv + out_proj_b) + b1[r]
    W2cat   = W2.transpose(0,2,1).reshape(R*DH, DOUT)
    out     = (gate*h_flat) @ W2cat + gate @ b2

Device (per core, batch-sharded 8 ways, 2048 rows each; all in feature-major
"T" layout so both GEMMs chain without transposes):
    gate phase: logitsT[8,b] -> E=exp(+bg) -> S=1@E -> rec -> gate_bf[8,b]
    main loop per 512-col batch chunk:
      GEMM1: psum[h,b] = sum_k w1tT[k,h-tile] x xT[k,b]   (bf16 MMs)
      evict: ACT relu(+b1f) -> f32 tmp; DVE tmp*gate_bcast -> bf16 Hg
      GEMM2: psum[o,b] = sum_k2 w2[k2,o-tile] x Hg[k2,b] + b2 x gate_bf
      evict: DVE copy -> f32 -> DMA outT
"""

import numpy as np

B, D, DOUT, R = 16384, 1024, 1024, 8
DH = D // 2            # 512
RH = R * DH            # 4096
NCORES = 8
BS = B // NCORES       # 2048 rows per core
CHUNK = 512
NCHUNK = BS // CHUNK   # 4
KT = D // 128          # 8 k-tiles over D
HT = RH // 128         # 32 h-tiles
K2T = RH // 128        # 32 k-tiles over RH
OT = DOUT // 128       # 8 out-tiles
GRP = DH // 128        # 4 h-tiles per route

_NC_CACHE = {}


def _build_nc(mm_dt_name="bfloat16"):
    from contextlib import ExitStack

    import concourse.bass as bass
    import concourse.mybir as mybir
    import concourse.tile as tile
    from concourse import bacc

    mm_dt = getattr(mybir.dt, mm_dt_name)
    f32 = mybir.dt.float32
    AF = mybir.ActivationFunctionType

    nc = bacc.Bacc("TRN2", target_bir_lowering=False, debug=False,
                   num_devices=NCORES)

    xT = nc.dram_tensor("xT", [KT, 128, BS], mm_dt, kind="ExternalInput")
    w1t = nc.dram_tensor("w1t", [KT, 128, RH], mm_dt, kind="ExternalInput")
    b1v = nc.dram_tensor("b1v", [128, HT], f32, kind="ExternalInput")
    w2 = nc.dram_tensor("w2", [OT, 128, RH], mm_dt, kind="ExternalInput")
    b2d = nc.dram_tensor("b2d", [R, DOUT], mm_dt, kind="ExternalInput")
    wgt = nc.dram_tensor("wgt", [KT, 128, R], mm_dt, kind="ExternalInput")
    bgd = nc.dram_tensor("bgd", [R, 1], f32, kind="ExternalInput")
    outT = nc.dram_tensor("outT", [OT, 128, BS], f32, kind="ExternalOutput")

    with tile.TileContext(nc) as tc, ExitStack() as ctx:
        const = ctx.enter_context(tc.tile_pool(name="const", bufs=1))

        # small consts first so the gate phase isn't stuck behind bulk DMA
        bg_sb = const.tile([R, 1], f32, tag="bg")
        nc.sync.dma_start(bg_sb[:], bgd[:, :])
        ones8b = const.tile([R, 1], mm_dt, tag="ones8b")
        nc.any.memset(ones8b[:], 1.0)
        ones1b = const.tile([1, 128], mm_dt, tag="ones1b")
        nc.any.memset(ones1b[:], 1.0)
        gate_bf = const.tile([R, BS], mm_dt, tag="gatebf")  # unnormalized exp(logits)
        ones1fp = const.tile([1, 128], f32, tag="ones1fp")
        nc.any.memset(ones1fp[:], 1.0)
        wg_sb = []
        for k in range(KT):
            t = const.tile([128, R], mm_dt, tag=f"wg_{k}", name=f"wgsb{k}")
            nc.sync.dma_start(t[:], wgt[k, :, :])
            wg_sb.append(t)

        xp = ctx.enter_context(tc.tile_pool(name="xp", bufs=2))
        growp = ctx.enter_context(tc.tile_pool(name="growp", bufs=2))
        gm = ctx.enter_context(tc.tile_pool(name="gm", bufs=2))
        gbcp = ctx.enter_context(tc.tile_pool(name="gbcp", bufs=2))
        srecp = ctx.enter_context(tc.tile_pool(name="srecp", bufs=2))
        hgp = ctx.enter_context(tc.tile_pool(name="hgp", bufs=1))
        tmpp = ctx.enter_context(tc.tile_pool(name="tmpp", bufs=3))
        w2p = ctx.enter_context(tc.tile_pool(name="w2p", bufs=3))
        outp = ctx.enter_context(tc.tile_pool(name="outp", bufs=3))
        p1 = ctx.enter_context(tc.tile_pool(name="p1", bufs=3, space="PSUM"))
        p2 = ctx.enter_context(tc.tile_pool(name="p2", bufs=2, space="PSUM"))
        pbc = ctx.enter_context(tc.tile_pool(name="pbc", bufs=3, space="PSUM"))

        xtiles = {}
        w1_last = [None]  # last w1 DMA inst; gates early competing DMAs

        def emit_x_prefetch(c):
            sl = slice(c * CHUNK, (c + 1) * CHUNK)
            xtiles[c] = []
            for k in range(KT):
                xt = xp.tile([128, CHUNK], mm_dt, tag=f"xt{k}",
                             name=f"xt{k}_{c}")
                dma = nc.sync.dma_start(xt[:], xT[k, :, sl])
                if c == 1 and w1_last[0] is not None:
                    tile.add_dep_helper(dma.ins, w1_last[0],
                                        reason="x(1) after w1 bulk load")
                xtiles[c].append(xt)

        gbcs = {}
        grows = {}
        recs = {}
        srecs = {}

        def emit_gate_logits(c):
            """E = exp(x@Wg.T + bg) for chunk c -> gate_bf[:, c] (bf16,
            unnormalized); the 1/sum factor is applied at GEMM2 eviction."""
            sl = slice(c * CHUNK, (c + 1) * CHUNK)
            pg = pbc.tile([R, CHUNK], f32, tag="pb", name=f"pg_{c}")
            for k in range(KT):
                nc.tensor.matmul(pg[:], wg_sb[k][:], xtiles[c][k][:],
                                 start=(k == 0), stop=(k == KT - 1))
            E = gm.tile([R, CHUNK], f32, tag="E", name=f"E_{c}")
            nc.scalar.activation(E[:], pg[:], AF.Exp, bias=bg_sb[:])
            nc.vector.tensor_copy(gate_bf[:, sl], E[:])
            # stage each row at partition 0 for the broadcast matmuls
            grows[c] = []
            for r in range(R):
                grow = growp.tile([1, CHUNK], mm_dt, tag=f"grow{r}",
                                  name=f"grow{r}_{c}")
                nc.sync.dma_start(grow[:], gate_bf[r:r + 1, sl])
                grows[c].append(grow)

        def emit_gate_bcast_rows(c, rs):
            """E rows -> 128-partition tiles via K=1 matmul."""
            gbcs.setdefault(c, [])
            for r in rs:
                pb = pbc.tile([128, CHUNK], f32, tag="pb", name=f"pb{r}_{c}")
                nc.tensor.matmul(pb[:], ones1b[:], grows[c][r][:],
                                 start=True, stop=True)
                g = gbcp.tile([128, CHUNK], mm_dt, tag=f"gbc{r}",
                              name=f"gbc{r}_{c}")
                nc.vector.tensor_copy(g[:], pb[:])
                gbcs[c].append(g)

        def emit_gate_sum(c):
            """S = sum_r E -> 1/S."""
            sl = slice(c * CHUNK, (c + 1) * CHUNK)
            ps = pbc.tile([1, CHUNK], f32, tag="pb", name=f"ps_{c}")
            nc.tensor.matmul(ps[:], ones8b[:], gate_bf[:, sl],
                             start=True, stop=True)
            rec = gm.tile([1, CHUNK], f32, tag="rec", name=f"rec_{c}")
            nc.vector.reciprocal(rec[:], ps[:])
            recs[c] = rec

        def emit_gate_bcast(c):
            emit_gate_bcast_rows(c, range(R))
            emit_gate_sum(c)

        def emit_srec_bcast(c):
            """broadcast 1/S to 128 partitions: hi/lo bf16 split keeps the
            K=1 matmuls single-pass while preserving ~fp32 precision."""
            rec = recs[c]
            rhi = gm.tile([1, CHUNK], mm_dt, tag="rhi", name=f"rhi_{c}")
            nc.vector.tensor_copy(rhi[:], rec[:])
            rlo = gm.tile([1, CHUNK], mm_dt, tag="rlo", name=f"rlo_{c}")
            nc.vector.tensor_sub(rlo[:], rec[:], rhi[:])
            pq = pbc.tile([128, CHUNK], f32, tag="pb", name=f"pq_{c}")
            nc.tensor.matmul(pq[:], ones1b[:], rhi[:], start=True, stop=False)
            nc.tensor.matmul(pq[:], ones1b[:], rlo[:], start=False, stop=True)
            srec = srecp.tile([128, CHUNK], f32, tag="srec",
                              name=f"srec_{c}")
            nc.vector.tensor_copy(srec[:], pq[:])
            srecs[c] = srec

        # prologue: x for chunk 0, gate(0) logits, then the bulk weight
        # loads. The bcast/srec matmuls are interleaved into chunk-0's
        # first GEMM1 groups to fill the w1-DMA-arrival bubbles.
        emit_x_prefetch(0)
        emit_gate_logits(0)

        w1_sb = []
        for k in range(KT):
            t = const.tile([128, RH], mm_dt, tag=f"w1_{k}", name=f"w1sb{k}")
            # two half-loads: finer arrival granularity while GEMM1(0) runs
            nc.sync.dma_start(t[:, :RH // 2], w1t[k, :, :RH // 2])
            dma = nc.sync.dma_start(t[:, RH // 2:], w1t[k, :, RH // 2:])
            w1_sb.append(t)
        w1_last[0] = dma.ins
        b1_sb = const.tile([128, HT], f32, tag="b1")
        nc.sync.dma_start(b1_sb[:], b1v[:, :])
        b2_sb = const.tile([R, DOUT], mm_dt, tag="b2")
        nc.sync.dma_start(b2_sb[:], b2d[:, :])

        for c in range(NCHUNK):
            sl = slice(c * CHUNK, (c + 1) * CHUNK)
            xts = xtiles.pop(c)
            if c + 1 < NCHUNK:
                emit_x_prefetch(c + 1)

            def evict_h(ht, ps1, hgs):
                tmp = tmpp.tile([128, CHUNK], f32, tag="tmp", name=f"tmp_{c}_{ht}")
                nc.scalar.activation(tmp[:], ps1[:], AF.Relu,
                                     bias=b1_sb[:, ht:ht + 1])
                hg = hgp.tile([128, CHUNK], mm_dt, tag=f"hg{ht}",
                              name=f"hg{ht}_{c}")
                nc.vector.tensor_mul(hg[:], tmp[:], gbcs[c][ht // GRP][:])
                hgs.append(hg)

            hgs = []
            if c == 0:
                # While w1 streams in, run ht groups pairwise with
                # interleaved k-loops (2 MMs ready per w1[k] arrival) and
                # slot the gate-aux matmuls into the remaining bubbles.
                for hp in range(4):
                    ha, hb = 2 * hp, 2 * hp + 1
                    pa = p1.tile([128, CHUNK], f32, tag="ps1", name=f"pa_{hp}")
                    pb2 = p1.tile([128, CHUNK], f32, tag="ps1", name=f"pb2_{hp}")
                    for k in range(KT):
                        nc.tensor.matmul(pa[:],
                                         w1_sb[k][:, ha * 128:(ha + 1) * 128],
                                         xts[k][:],
                                         start=(k == 0), stop=(k == KT - 1))
                        if hp == 0 and k >= 4:
                            emit_gate_bcast_rows(0, [2 * (k - 4)])
                        elif hp == 1 and k == 2:
                            emit_srec_bcast(0)
                        nc.tensor.matmul(pb2[:],
                                         w1_sb[k][:, hb * 128:(hb + 1) * 128],
                                         xts[k][:],
                                         start=(k == 0), stop=(k == KT - 1))
                        if hp == 0 and k >= 4:
                            emit_gate_bcast_rows(0, [2 * (k - 4) + 1])
                    if hp == 0:
                        emit_gate_sum(0)
                    evict_h(ha, pa, hgs)
                    evict_h(hb, pb2, hgs)
                ht_start = 8
            else:
                ht_start = 0

            for ht in range(ht_start, HT):
                ps1 = p1.tile([128, CHUNK], f32, tag="ps1")
                for k in range(KT):
                    nc.tensor.matmul(ps1[:],
                                     w1_sb[k][:, ht * 128:(ht + 1) * 128],
                                     xts[k][:],
                                     start=(k == 0), stop=(k == KT - 1))
                evict_h(ht, ps1, hgs)
                # next chunk's gate chain, scattered so cross-engine latency
                # hides behind GEMM1 groups instead of stalling in-order PE
                if c + 1 < NCHUNK:
                    if ht == 10:
                        emit_gate_logits(c + 1)
                    elif ht == 18:
                        emit_gate_bcast(c + 1)
                    elif ht == 26:
                        emit_srec_bcast(c + 1)

            for ot in range(OT):
                w2t = w2p.tile([128, RH], mm_dt, tag="w2t")
                dma = nc.sync.dma_start(w2t[:], w2[ot, :, :])
                if c == 0:
                    tile.add_dep_helper(dma.ins, w1_last[0],
                                        reason="w2 stream after w1 bulk load")
                ps2 = p2.tile([128, CHUNK], f32, tag="ps2")
                for k2 in range(K2T):
                    nc.tensor.matmul(ps2[:],
                                     w2t[:, k2 * 128:(k2 + 1) * 128],
                                     hgs[k2][:],
                                     start=(k2 == 0), stop=False)
                nc.tensor.matmul(ps2[:], b2_sb[:, ot * 128:(ot + 1) * 128],
                                 gate_bf[:, sl], start=False, stop=True)
                osb = outp.tile([128, CHUNK], f32, tag="osb")
                nc.vector.tensor_mul(osb[:], ps2[:], srecs[c][:])
                nc.sync.dma_start(outT[ot, :, sl], osb[:])
            del gbcs[c], srecs[c], grows[c], recs[c]

    nc.compile()
    return nc


def _get_nc(mm_dt_name="bfloat16"):
    if mm_dt_name not in _NC_CACHE:
        _NC_CACHE[mm_dt_name] = _build_nc(mm_dt_name)
    return _NC_CACHE[mm_dt_name]


def _prepare_in_maps(inputs, np_mm_dtype):
    x = np.asarray(inputs["x"], np.float32)
    in_proj_w = np.asarray(inputs["in_proj_w"], np.float32)
    in_proj_b = np.asarray(inputs["in_proj_b"], np.float32)
    out_proj_w = np.asarray(inputs["out_proj_w"], np.float32)
    out_proj_b = np.asarray(inputs["out_proj_b"], np.float32)
    W1 = np.asarray(inputs["W1"], np.float32)
    b1 = np.asarray(inputs["b1"], np.float32)
    W2 = np.asarray(inputs["W2"], np.float32)
    b2 = np.asarray(inputs["b2"], np.float32)
    Wg = np.asarray(inputs["Wg"], np.float32)
    bg = np.asarray(inputs["bg"], np.float32)

    Wv = in_proj_w[2 * D:]
    bv = in_proj_b[2 * D:]
    A = out_proj_w @ Wv                       # [D, D]
    ba = out_proj_w @ bv + out_proj_b         # [D]
    W1r = W1.reshape(RH, D)
    W1f = W1r @ A                             # [RH, D]
    b1f = W1r @ ba + b1.reshape(RH)           # [RH]
    W2cat = W2.transpose(0, 2, 1).reshape(RH, DOUT)

    w1t_np = np.ascontiguousarray(W1f.T).reshape(KT, 128, RH)
    b1v_np = np.ascontiguousarray(b1f.reshape(HT, 128).T)
    w2_np = np.ascontiguousarray(
        W2cat.reshape(K2T, 128, OT, 128).transpose(2, 1, 0, 3)
    ).reshape(OT, 128, RH)
    wgt_np = np.ascontiguousarray(Wg.T).reshape(KT, 128, R)
    bg_np = np.ascontiguousarray(bg.reshape(R, 1))

    shared = {
        "w1t": w1t_np.astype(np_mm_dtype),
        "b1v": b1v_np,
        "w2": w2_np.astype(np_mm_dtype),
        "b2d": b2.astype(np_mm_dtype),
        "wgt": wgt_np.astype(np_mm_dtype),
        "bgd": bg_np,
    }
    in_maps = []
    for c in range(NCORES):
        xs = x[c * BS:(c + 1) * BS]           # [BS, D]
        xT_np = np.ascontiguousarray(xs.T).reshape(KT, 128, BS)
        m = dict(shared)
        m["xT"] = xT_np.astype(np_mm_dtype)
        in_maps.append(m)
    return in_maps


def _run(inputs, trace=False, mm_dt_name="bfloat16"):
    import ml_dtypes
    from concourse.bass_utils import run_bass_kernel_spmd

    np_mm = ml_dtypes.bfloat16 if mm_dt_name == "bfloat16" else np.float32
    nc = _get_nc(mm_dt_name)
    in_maps = _prepare_in_maps(inputs, np_mm)
    res = run_bass_kernel_spmd(nc, in_maps, list(range(NCORES)), trace=trace)
    out = np.empty((B, DOUT), np.float32)
    for c in range(NCORES):
        out[c * BS:(c + 1) * BS] = res.results[c]["outT"].reshape(DOUT, BS).T
    return out, res


def kernel(**inputs):
    out, _ = _run(inputs, trace=False)
    return out


# revision 29
# speedup vs baseline: 1.0295x; 1.0295x over previous
"""Trainium2 Bass kernel for nn_BrainInspiredRouter.

Math (reference, seq_len==1 attention => attn collapses to the V path):
    attended = x @ (out_proj_w @ Wv).T + (out_proj_w @ bv + out_proj_b)
    h        = relu(attended @ W1[r].T + b1[r])          per route r
    route    = h @ W2[r].T + b2[r]
    gate     = softmax(x @ Wg.T + bg)
    out      = sum_r gate[:, r] * route[:, r, :]

Host-side constant folding (weights only, no activations):
    W1f[r]  = W1[r] @ (out_proj_w @ Wv)      -> h = relu(x @ W1f.T + b1f)
    b1f[r]  = W1[r] @ (out_proj_w@bv + out_proj_b) + b1[r]
    W2cat   = W2.transpose(0,2,1).reshape(R*DH, DOUT)
    out     = (gate*h_flat) @ W2cat + gate @ b2

Device (per core, batch-sharded 8 ways, 2048 rows each; all in feature-major
"T" layout so both GEMMs chain without transposes):
    gate phase: logitsT[8,b] -> E=exp(+bg) -> S=1@E -> rec -> gate_bf[8,b]
    main loop per 512-col batch chunk:
      GEMM1: psum[h,b] = sum_k w1tT[k,h-tile] x xT[k,b]   (bf16 MMs)
      evict: ACT relu(+b1f) -> f32 tmp; DVE tmp*gate_bcast -> bf16 Hg
      GEMM2: psum[o,b] = sum_k2 w2[k2,o-tile] x Hg[k2,b] + b2 x gate_bf
      evict: DVE copy -> f32 -> DMA outT
"""

import numpy as np

B, D, DOUT, R = 16384, 1024, 1024, 8
DH = D // 2            # 512
RH = R * DH            # 4096
NCORES = 8
BS = B // NCORES       # 2048 rows per core
CHUNK = 512
NCHUNK = BS // CHUNK   # 4
KT = D // 128          # 8 k-tiles over D
HT = RH // 128         # 32 h-tiles
K2T = RH // 128        # 32 k-tiles over RH
OT = DOUT // 128       # 8 out-tiles
GRP = DH // 128        # 4 h-tiles per route

_NC_CACHE = {}


def _build_nc(mm_dt_name="bfloat16"):
    from contextlib import ExitStack

    import concourse.bass as bass
    import concourse.mybir as mybir
    import concourse.tile as tile
    from concourse import bacc

    mm_dt = getattr(mybir.dt, mm_dt_name)
    f32 = mybir.dt.float32
    AF = mybir.ActivationFunctionType

    nc = bacc.Bacc("TRN2", target_bir_lowering=False, debug=False,
                   num_devices=NCORES)

    xT = nc.dram_tensor("xT", [KT, 128, BS], mm_dt, kind="ExternalInput")
    w1t = nc.dram_tensor("w1t", [KT, 128, RH], mm_dt, kind="ExternalInput")
    b1v = nc.dram_tensor("b1v", [128, HT], f32, kind="ExternalInput")
    w2 = nc.dram_tensor("w2", [OT, 128, RH], mm_dt, kind="ExternalInput")
    b2d = nc.dram_tensor("b2d", [R, DOUT], mm_dt, kind="ExternalInput")
    wgt = nc.dram_tensor("wgt", [KT, 128, R], mm_dt, kind="ExternalInput")
    bgd = nc.dram_tensor("bgd", [R, 1], f32, kind="ExternalInput")
    outT = nc.dram_tensor("outT", [OT, 128, BS], f32, kind="ExternalOutput")
    gate_scr = nc.dram_tensor("gate_scr", [R, BS], mm_dt)
    srec_scr = nc.dram_tensor("srec_scr", [1, BS], f32)

    with tile.TileContext(nc) as tc, ExitStack() as ctx:
        const = ctx.enter_context(tc.tile_pool(name="const", bufs=1))

        # small consts first so the gate phase isn't stuck behind bulk DMA
        bg_sb = const.tile([R, 1], f32, tag="bg")
        nc.sync.dma_start(bg_sb[:], bgd[:, :])
        ones8b = const.tile([R, 1], mm_dt, tag="ones8b")
        nc.any.memset(ones8b[:], 1.0)
        gate_bf = const.tile([R, BS], mm_dt, tag="gatebf")  # unnormalized exp(logits)
        wg_sb = []
        for k in range(KT):
            t = const.tile([128, R], mm_dt, tag=f"wg_{k}", name=f"wgsb{k}")
            nc.sync.dma_start(t[:], wgt[k, :, :])
            wg_sb.append(t)

        xp = ctx.enter_context(tc.tile_pool(name="xp", bufs=2))
        gm = ctx.enter_context(tc.tile_pool(name="gm", bufs=2))
        gbcp = ctx.enter_context(tc.tile_pool(name="gbcp", bufs=2))
        srecp = ctx.enter_context(tc.tile_pool(name="srecp", bufs=2))
        hgp = ctx.enter_context(tc.tile_pool(name="hgp", bufs=1))
        tmpp = ctx.enter_context(tc.tile_pool(name="tmpp", bufs=3))
        w2p = ctx.enter_context(tc.tile_pool(name="w2p", bufs=3))
        outp = ctx.enter_context(tc.tile_pool(name="outp", bufs=3))
        p1 = ctx.enter_context(tc.tile_pool(name="p1", bufs=4, space="PSUM"))
        p2 = ctx.enter_context(tc.tile_pool(name="p2", bufs=2, space="PSUM"))
        pbc = ctx.enter_context(tc.tile_pool(name="pbc", bufs=2, space="PSUM"))

        xtiles = {}
        w1_last = [None]  # last w1 DMA inst; gates early competing DMAs

        def emit_x_prefetch(c):
            sl = slice(c * CHUNK, (c + 1) * CHUNK)
            xtiles[c] = []
            for k in range(KT):
                xt = xp.tile([128, CHUNK], mm_dt, tag=f"xt{k}",
                             name=f"xt{k}_{c}")
                dma = nc.sync.dma_start(xt[:], xT[k, :, sl])
                if c == 1 and w1_last[0] is not None:
                    tile.add_dep_helper(dma.ins, w1_last[0],
                                        reason="x(1) after w1 bulk load")
                xtiles[c].append(xt)

        gbcs = {}
        grows = {}
        recs = {}
        srecs = {}

        def emit_gate_logits(c):
            """E = exp(x@Wg.T + bg) for chunk c -> gate_bf[:, c] (bf16,
            unnormalized); the 1/sum factor is applied at GEMM2 eviction."""
            sl = slice(c * CHUNK, (c + 1) * CHUNK)
            pg = pbc.tile([R, CHUNK], f32, tag="pb", name=f"pg_{c}")
            for k in range(KT):
                nc.tensor.matmul(pg[:], wg_sb[k][:], xtiles[c][k][:],
                                 start=(k == 0), stop=(k == KT - 1))
            E = gm.tile([R, CHUNK], f32, tag="E", name=f"E_{c}")
            nc.scalar.activation(E[:], pg[:], AF.Exp, bias=bg_sb[:])
            nc.vector.tensor_copy(gate_bf[:, sl], E[:])
            # round-trip through DRAM so DMA can replicate rows across
            # partitions (step-0 source AP); dep edges added explicitly
            grows[c] = nc.sync.dma_start(gate_scr[:, sl], gate_bf[:, sl])

        def emit_gate_bcast_rows(c, rs):
            """E rows -> 128-partition tiles via replicating DMA."""
            gbcs.setdefault(c, [])
            for r in rs:
                g = gbcp.tile([128, CHUNK], mm_dt, tag=f"gbc{r}",
                              name=f"gbc{r}_{c}")
                src = bass.AP(gate_scr, r * BS + c * CHUNK,
                              [[0, 128], [1, CHUNK]])
                dma = nc.sync.dma_start(g[:], src)
                tile.add_dep_helper(dma.ins, grows[c].ins,
                                    reason="gate bcast read after scr write")
                gbcs[c].append(g)

        def emit_gate_sum(c):
            """S = sum_r E -> 1/S."""
            sl = slice(c * CHUNK, (c + 1) * CHUNK)
            ps = pbc.tile([1, CHUNK], f32, tag="pb", name=f"ps_{c}")
            nc.tensor.matmul(ps[:], ones8b[:], gate_bf[:, sl],
                             start=True, stop=True)
            rec = gm.tile([1, CHUNK], f32, tag="rec", name=f"rec_{c}")
            nc.vector.reciprocal(rec[:], ps[:])
            recs[c] = rec

        def emit_gate_bcast(c):
            emit_gate_bcast_rows(c, range(R))
            emit_gate_sum(c)

        def emit_srec_bcast(c):
            """broadcast 1/S to 128 partitions via replicating DMA."""
            sl = slice(c * CHUNK, (c + 1) * CHUNK)
            w = nc.sync.dma_start(srec_scr[:, sl], recs[c][:])
            srec = srecp.tile([128, CHUNK], f32, tag="srec",
                              name=f"srec_{c}")
            src = bass.AP(srec_scr, c * CHUNK, [[0, 128], [1, CHUNK]])
            dma = nc.sync.dma_start(srec[:], src)
            tile.add_dep_helper(dma.ins, w.ins,
                                reason="srec bcast read after scr write")
            srecs[c] = srec

        # prologue: x for chunk 0, gate(0) logits, then the bulk weight
        # loads. The bcast/srec matmuls are interleaved into chunk-0's
        # first GEMM1 groups to fill the w1-DMA-arrival bubbles.
        emit_x_prefetch(0)
        emit_gate_logits(0)

        w1_sb = []
        for k in range(KT):
            t = const.tile([128, RH], mm_dt, tag=f"w1_{k}", name=f"w1sb{k}")
            # two half-loads: finer arrival granularity while GEMM1(0) runs
            nc.sync.dma_start(t[:, :RH // 2], w1t[k, :, :RH // 2])
            dma = nc.sync.dma_start(t[:, RH // 2:], w1t[k, :, RH // 2:])
            w1_sb.append(t)
        w1_last[0] = dma.ins
        b1_sb = const.tile([128, HT], f32, tag="b1")
        nc.sync.dma_start(b1_sb[:], b1v[:, :])
        b2_sb = const.tile([R, DOUT], mm_dt, tag="b2")
        nc.sync.dma_start(b2_sb[:], b2d[:, :])

        for c in range(NCHUNK):
            sl = slice(c * CHUNK, (c + 1) * CHUNK)
            xts = xtiles.pop(c)
            if c + 1 < NCHUNK:
                emit_x_prefetch(c + 1)

            def evict_h(ht, ps1, hgs):
                tmp = tmpp.tile([128, CHUNK], f32, tag="tmp", name=f"tmp_{c}_{ht}")
                nc.scalar.activation(tmp[:], ps1[:], AF.Relu,
                                     bias=b1_sb[:, ht:ht + 1])
                hg = hgp.tile([128, CHUNK], mm_dt, tag=f"hg{ht}",
                              name=f"hg{ht}_{c}")
                nc.vector.tensor_mul(hg[:], tmp[:], gbcs[c][ht // GRP][:])
                hgs.append(hg)

            hgs = []
            if c == 0:
                # While w1 streams in, run ht groups pairwise with
                # interleaved k-loops (2 MMs ready per w1[k] arrival) and
                # slot the gate-aux matmuls into the remaining bubbles.
                for hp in range(4):
                    ha, hb = 2 * hp, 2 * hp + 1
                    pa = p1.tile([128, CHUNK], f32, tag="ps1", name=f"pa_{hp}")
                    pb2 = p1.tile([128, CHUNK], f32, tag="ps1", name=f"pb2_{hp}")
                    for k in range(KT):
                        nc.tensor.matmul(pa[:],
                                         w1_sb[k][:, ha * 128:(ha + 1) * 128],
                                         xts[k][:],
                                         start=(k == 0), stop=(k == KT - 1))
                        if hp == 0 and k >= 4:
                            emit_gate_bcast_rows(0, [2 * (k - 4)])
                        elif hp == 1 and k == 2:
                            emit_srec_bcast(0)
                        nc.tensor.matmul(pb2[:],
                                         w1_sb[k][:, hb * 128:(hb + 1) * 128],
                                         xts[k][:],
                                         start=(k == 0), stop=(k == KT - 1))
                        if hp == 0 and k >= 4:
                            emit_gate_bcast_rows(0, [2 * (k - 4) + 1])
                    if hp == 0:
                        emit_gate_sum(0)
                    evict_h(ha, pa, hgs)
                    evict_h(hb, pb2, hgs)
                ht_start = 8
            else:
                ht_start = 0

            for ht in range(ht_start, HT):
                ps1 = p1.tile([128, CHUNK], f32, tag="ps1")
                for k in range(KT):
                    nc.tensor.matmul(ps1[:],
                                     w1_sb[k][:, ht * 128:(ht + 1) * 128],
                                     xts[k][:],
                                     start=(k == 0), stop=(k == KT - 1))
                evict_h(ht, ps1, hgs)
                # next chunk's gate chain, scattered so cross-engine latency
                # hides behind GEMM1 groups instead of stalling in-order PE
                if c + 1 < NCHUNK:
                    if ht == 10:
                        emit_gate_logits(c + 1)
                    elif ht == 18:
                        emit_gate_bcast(c + 1)
                    elif ht == 26:
                        emit_srec_bcast(c + 1)

            for ot in range(OT):
                w2t = w2p.tile([128, RH], mm_dt, tag="w2t")
                dma = nc.sync.dma_start(w2t[:], w2[ot, :, :])
                if c == 0:
                    tile.add_dep_helper(dma.ins, w1_last[0],
                                        reason="w2 stream after w1 bulk load")
                ps2 = p2.tile([128, CHUNK], f32, tag="ps2")
                for k2 in range(K2T):
                    nc.tensor.matmul(ps2[:],
                                     w2t[:, k2 * 128:(k2 + 1) * 128],
                                     hgs[k2][:],
                                     start=(k2 == 0), stop=False)
                nc.tensor.matmul(ps2[:], b2_sb[:, ot * 128:(ot + 1) * 128],
                                 gate_bf[:, sl], start=False, stop=True)
                osb = outp.tile([128, CHUNK], f32, tag="osb")
                nc.vector.tensor_mul(osb[:], ps2[:], srecs[c][:])
                nc.sync.dma_start(outT[ot, :, sl], osb[:])
            del gbcs[c], srecs[c], grows[c], recs[c]

    nc.compile()
    return nc


def _get_nc(mm_dt_name="bfloat16"):
    if mm_dt_name not in _NC_CACHE:
        _NC_CACHE[mm_dt_name] = _build_nc(mm_dt_name)
    return _NC_CACHE[mm_dt_name]


def _prepare_in_maps(inputs, np_mm_dtype):
    x = np.asarray(inputs["x"], np.float32)
    in_proj_w = np.asarray(inputs["in_proj_w"], np.float32)
    in_proj_b = np.asarray(inputs["in_proj_b"], np.float32)
    out_proj_w = np.asarray(inputs["out_proj_w"], np.float32)
    out_proj_b = np.asarray(inputs["out_proj_b"], np.float32)
    W1 = np.asarray(inputs["W1"], np.float32)
    b1 = np.asarray(inputs["b1"], np.float32)
    W2 = np.asarray(inputs["W2"], np.float32)
    b2 = np.asarray(inputs["b2"], np.float32)
    Wg = np.asarray(inputs["Wg"], np.float32)
    bg = np.asarray(inputs["bg"], np.float32)

    Wv = in_proj_w[2 * D:]
    bv = in_proj_b[2 * D:]
    A = out_proj_w @ Wv                       # [D, D]
    ba = out_proj_w @ bv + out_proj_b         # [D]
    W1r = W1.reshape(RH, D)
    W1f = W1r @ A                             # [RH, D]
    b1f = W1r @ ba + b1.reshape(RH)           # [RH]
    W2cat = W2.transpose(0, 2, 1).reshape(RH, DOUT)

    w1t_np = np.ascontiguousarray(W1f.T).reshape(KT, 128, RH)
    b1v_np = np.ascontiguousarray(b1f.reshape(HT, 128).T)
    w2_np = np.ascontiguousarray(
        W2cat.reshape(K2T, 128, OT, 128).transpose(2, 1, 0, 3)
    ).reshape(OT, 128, RH)
    wgt_np = np.ascontiguousarray(Wg.T).reshape(KT, 128, R)
    bg_np = np.ascontiguousarray(bg.reshape(R, 1))

    shared = {
        "w1t": w1t_np.astype(np_mm_dtype),
        "b1v": b1v_np,
        "w2": w2_np.astype(np_mm_dtype),
        "b2d": b2.astype(np_mm_dtype),
        "wgt": wgt_np.astype(np_mm_dtype),
        "bgd": bg_np,
    }
    in_maps = []
    for c in range(NCORES):
        xs = x[c * BS:(c + 1) * BS]           # [BS, D]
        xT_np = np.ascontiguousarray(xs.T).reshape(KT, 128, BS)
        m = dict(shared)
        m["xT"] = xT_np.astype(np_mm_dtype)
        in_maps.append(m)
    return in_maps


def _run(inputs, trace=False, mm_dt_name="bfloat16"):
    import ml_dtypes
    from concourse.bass_utils import run_bass_kernel_spmd

    np_mm = ml_dtypes.bfloat16 if mm_dt_name == "bfloat16" else np.float32
    nc = _get_nc(mm_dt_name)
    in_maps = _prepare_in_maps(inputs, np_mm)
    res = run_bass_kernel_spmd(nc, in_maps, list(range(NCORES)), trace=trace)
    out = np.empty((B, DOUT), np.float32)
    for c in range(NCORES):
        out[c * BS:(c + 1) * BS] = res.results[c]["outT"].reshape(DOUT, BS).T
    return out, res


def kernel(**inputs):
    out, _ = _run(inputs, trace=False)
    return out


# revision 30
# speedup vs baseline: 1.0356x; 1.0059x over previous
"""Trainium2 Bass kernel for nn_BrainInspiredRouter.

Math (reference, seq_len==1 attention => attn collapses to the V path):
    attended = x @ (out_proj_w @ Wv).T + (out_proj_w @ bv + out_proj_b)
    h        = relu(attended @ W1[r].T + b1[r])          per route r
    route    = h @ W2[r].T + b2[r]
    gate     = softmax(x @ Wg.T + bg)
    out      = sum_r gate[:, r] * route[:, r, :]

Host-side constant folding (weights only, no activations):
    W1f[r]  = W1[r] @ (out_proj_w @ Wv)      -> h = relu(x @ W1f.T + b1f)
    b1f[r]  = W1[r] @ (out_proj_w@bv + out_proj_b) + b1[r]
    W2cat   = W2.transpose(0,2,1).reshape(R*DH, DOUT)
    out     = (gate*h_flat) @ W2cat + gate @ b2

Device (per core, batch-sharded 8 ways, 2048 rows each; all in feature-major
"T" layout so both GEMMs chain without transposes):
    gate phase: logitsT[8,b] -> E=exp(+bg) -> S=1@E -> rec -> gate_bf[8,b]
    main loop per 512-col batch chunk:
      GEMM1: psum[h,b] = sum_k w1tT[k,h-tile] x xT[k,b]   (bf16 MMs)
      evict: ACT relu(+b1f) -> f32 tmp; DVE tmp*gate_bcast -> bf16 Hg
      GEMM2: psum[o,b] = sum_k2 w2[k2,o-tile] x Hg[k2,b] + b2 x gate_bf
      evict: DVE copy -> f32 -> DMA outT
"""

import numpy as np

B, D, DOUT, R = 16384, 1024, 1024, 8
DH = D // 2            # 512
RH = R * DH            # 4096
NCORES = 8
BS = B // NCORES       # 2048 rows per core
CHUNK = 512
NCHUNK = BS // CHUNK   # 4
KT = D // 128          # 8 k-tiles over D
HT = RH // 128         # 32 h-tiles
K2T = RH // 128        # 32 k-tiles over RH
OT = DOUT // 128       # 8 out-tiles
GRP = DH // 128        # 4 h-tiles per route

_NC_CACHE = {}


def _build_nc(mm_dt_name="bfloat16"):
    from contextlib import ExitStack

    import concourse.bass as bass
    import concourse.mybir as mybir
    import concourse.tile as tile
    from concourse import bacc

    mm_dt = getattr(mybir.dt, mm_dt_name)
    f32 = mybir.dt.float32
    AF = mybir.ActivationFunctionType

    nc = bacc.Bacc("TRN2", target_bir_lowering=False, debug=False,
                   num_devices=NCORES)

    xT = nc.dram_tensor("xT", [KT, 128, BS], mm_dt, kind="ExternalInput")
    w1t = nc.dram_tensor("w1t", [KT, 128, RH], mm_dt, kind="ExternalInput")
    b1v = nc.dram_tensor("b1v", [128, HT], f32, kind="ExternalInput")
    w2 = nc.dram_tensor("w2", [OT, 128, RH], mm_dt, kind="ExternalInput")
    b2d = nc.dram_tensor("b2d", [R, DOUT], mm_dt, kind="ExternalInput")
    wgt = nc.dram_tensor("wgt", [KT, 128, R], mm_dt, kind="ExternalInput")
    bgd = nc.dram_tensor("bgd", [R, 1], f32, kind="ExternalInput")
    outT = nc.dram_tensor("outT", [OT, 128, BS], f32, kind="ExternalOutput")
    gate_scr = nc.dram_tensor("gate_scr", [R, BS], mm_dt)
    srec_scr = nc.dram_tensor("srec_scr", [1, BS], f32)

    with tile.TileContext(nc) as tc, ExitStack() as ctx:
        const = ctx.enter_context(tc.tile_pool(name="const", bufs=1))

        # small consts first so the gate phase isn't stuck behind bulk DMA
        bg_sb = const.tile([R, 1], f32, tag="bg")
        nc.sync.dma_start(bg_sb[:], bgd[:, :])
        ones8b = const.tile([R, 1], mm_dt, tag="ones8b")
        nc.any.memset(ones8b[:], 1.0)
        gate_bf = const.tile([R, BS], mm_dt, tag="gatebf")  # unnormalized exp(logits)
        wg_sb = []
        for k in range(KT):
            t = const.tile([128, R], mm_dt, tag=f"wg_{k}", name=f"wgsb{k}")
            nc.sync.dma_start(t[:], wgt[k, :, :])
            wg_sb.append(t)

        xp = ctx.enter_context(tc.tile_pool(name="xp", bufs=2))
        gm = ctx.enter_context(tc.tile_pool(name="gm", bufs=2))
        gbcp = ctx.enter_context(tc.tile_pool(name="gbcp", bufs=2))
        srecp = ctx.enter_context(tc.tile_pool(name="srecp", bufs=2))
        hgp = ctx.enter_context(tc.tile_pool(name="hgp", bufs=1))
        tmpp = ctx.enter_context(tc.tile_pool(name="tmpp", bufs=3))
        w2p = ctx.enter_context(tc.tile_pool(name="w2p", bufs=3))
        outp = ctx.enter_context(tc.tile_pool(name="outp", bufs=3))
        p1 = ctx.enter_context(tc.tile_pool(name="p1", bufs=4, space="PSUM"))
        p2 = ctx.enter_context(tc.tile_pool(name="p2", bufs=2, space="PSUM"))
        pbc = ctx.enter_context(tc.tile_pool(name="pbc", bufs=2, space="PSUM"))

        xtiles = {}
        w1_last = [None]  # last w1 DMA inst; gates early competing DMAs

        def emit_x_prefetch(c):
            sl = slice(c * CHUNK, (c + 1) * CHUNK)
            xtiles[c] = []
            for k in range(KT):
                xt = xp.tile([128, CHUNK], mm_dt, tag=f"xt{k}",
                             name=f"xt{k}_{c}")
                dma = nc.sync.dma_start(xt[:], xT[k, :, sl])
                if c == 1 and w1_last[0] is not None:
                    tile.add_dep_helper(dma.ins, w1_last[0],
                                        reason="x(1) after w1 bulk load")
                xtiles[c].append(xt)

        gbcs = {}
        grows = {}
        recs = {}
        srecs = {}

        def emit_gate_logits(c):
            """E = exp(x@Wg.T + bg) for chunk c -> gate_bf[:, c] (bf16,
            unnormalized); the 1/sum factor is applied at GEMM2 eviction."""
            sl = slice(c * CHUNK, (c + 1) * CHUNK)
            pg = pbc.tile([R, CHUNK], f32, tag="pb", name=f"pg_{c}")
            for k in range(KT):
                nc.tensor.matmul(pg[:], wg_sb[k][:], xtiles[c][k][:],
                                 start=(k == 0), stop=(k == KT - 1))
            E = gm.tile([R, CHUNK], f32, tag="E", name=f"E_{c}")
            nc.scalar.activation(E[:], pg[:], AF.Exp, bias=bg_sb[:])
            nc.vector.tensor_copy(gate_bf[:, sl], E[:])
            # round-trip through DRAM so DMA can replicate rows across
            # partitions (step-0 source AP); dep edges added explicitly
            grows[c] = nc.sync.dma_start(gate_scr[:, sl], gate_bf[:, sl])

        def emit_gate_bcast_rows(c, rs):
            """E rows -> 128-partition tiles via replicating DMA."""
            gbcs.setdefault(c, [])
            for r in rs:
                g = gbcp.tile([128, CHUNK], mm_dt, tag=f"gbc{r}",
                              name=f"gbc{r}_{c}")
                src = bass.AP(gate_scr, r * BS + c * CHUNK,
                              [[0, 128], [1, CHUNK]])
                dma = nc.sync.dma_start(g[:], src)
                tile.add_dep_helper(dma.ins, grows[c].ins,
                                    reason="gate bcast read after scr write")
                gbcs[c].append(g)

        def emit_gate_sum(c):
            """S = sum_r E -> 1/S."""
            sl = slice(c * CHUNK, (c + 1) * CHUNK)
            ps = pbc.tile([1, CHUNK], f32, tag="pb", name=f"ps_{c}")
            nc.tensor.matmul(ps[:], ones8b[:], gate_bf[:, sl],
                             start=True, stop=True)
            rec = gm.tile([1, CHUNK], f32, tag="rec", name=f"rec_{c}")
            nc.vector.reciprocal(rec[:], ps[:])
            recs[c] = rec

        def emit_gate_bcast(c):
            emit_gate_bcast_rows(c, range(R))
            emit_gate_sum(c)

        def emit_srec_bcast(c):
            """broadcast 1/S to 128 partitions via replicating DMA."""
            sl = slice(c * CHUNK, (c + 1) * CHUNK)
            w = nc.sync.dma_start(srec_scr[:, sl], recs[c][:])
            srec = srecp.tile([128, CHUNK], f32, tag="srec",
                              name=f"srec_{c}")
            src = bass.AP(srec_scr, c * CHUNK, [[0, 128], [1, CHUNK]])
            dma = nc.sync.dma_start(srec[:], src)
            tile.add_dep_helper(dma.ins, w.ins,
                                reason="srec bcast read after scr write")
            srecs[c] = srec

        # prologue: x for chunk 0, gate(0) logits, then the bulk weight
        # loads. The bcast/srec matmuls are interleaved into chunk-0's
        # first GEMM1 groups to fill the w1-DMA-arrival bubbles.
        emit_x_prefetch(0)
        emit_gate_logits(0)

        w1_sb = [const.tile([128, RH], mm_dt, tag=f"w1_{k}", name=f"w1sb{k}")
                 for k in range(KT)]
        # load in h-quarters, all k per quarter: GEMM1(0)'s first h-tiles
        # unblock after 2MB instead of the full 8MB
        Q = RH // 4
        for q in range(4):
            for k in range(KT):
                dma = nc.sync.dma_start(w1_sb[k][:, q * Q:(q + 1) * Q],
                                        w1t[k, :, q * Q:(q + 1) * Q])
        w1_last[0] = dma.ins
        b1_sb = const.tile([128, HT], f32, tag="b1")
        nc.sync.dma_start(b1_sb[:], b1v[:, :])
        b2_sb = const.tile([R, DOUT], mm_dt, tag="b2")
        nc.sync.dma_start(b2_sb[:], b2d[:, :])

        for c in range(NCHUNK):
            sl = slice(c * CHUNK, (c + 1) * CHUNK)
            xts = xtiles.pop(c)
            if c + 1 < NCHUNK:
                emit_x_prefetch(c + 1)

            def evict_h(ht, ps1, hgs):
                tmp = tmpp.tile([128, CHUNK], f32, tag="tmp", name=f"tmp_{c}_{ht}")
                nc.scalar.activation(tmp[:], ps1[:], AF.Relu,
                                     bias=b1_sb[:, ht:ht + 1])
                hg = hgp.tile([128, CHUNK], mm_dt, tag=f"hg{ht}",
                              name=f"hg{ht}_{c}")
                nc.vector.tensor_mul(hg[:], tmp[:], gbcs[c][ht // GRP][:])
                hgs.append(hg)

            hgs = []
            if c == 0:
                # While w1 streams in, run ht groups pairwise with
                # interleaved k-loops (2 MMs ready per w1[k] arrival) and
                # slot the gate-aux matmuls into the remaining bubbles.
                for hp in range(4):
                    ha, hb = 2 * hp, 2 * hp + 1
                    pa = p1.tile([128, CHUNK], f32, tag="ps1", name=f"pa_{hp}")
                    pb2 = p1.tile([128, CHUNK], f32, tag="ps1", name=f"pb2_{hp}")
                    for k in range(KT):
                        nc.tensor.matmul(pa[:],
                                         w1_sb[k][:, ha * 128:(ha + 1) * 128],
                                         xts[k][:],
                                         start=(k == 0), stop=(k == KT - 1))
                        if hp == 0 and k >= 4:
                            emit_gate_bcast_rows(0, [2 * (k - 4)])
                        elif hp == 1 and k == 2:
                            emit_srec_bcast(0)
                        nc.tensor.matmul(pb2[:],
                                         w1_sb[k][:, hb * 128:(hb + 1) * 128],
                                         xts[k][:],
                                         start=(k == 0), stop=(k == KT - 1))
                        if hp == 0 and k >= 4:
                            emit_gate_bcast_rows(0, [2 * (k - 4) + 1])
                    if hp == 0:
                        emit_gate_sum(0)
                    evict_h(ha, pa, hgs)
                    evict_h(hb, pb2, hgs)
                ht_start = 8
            else:
                ht_start = 0

            for ht in range(ht_start, HT):
                ps1 = p1.tile([128, CHUNK], f32, tag="ps1")
                for k in range(KT):
                    nc.tensor.matmul(ps1[:],
                                     w1_sb[k][:, ht * 128:(ht + 1) * 128],
                                     xts[k][:],
                                     start=(k == 0), stop=(k == KT - 1))
                evict_h(ht, ps1, hgs)
                # next chunk's gate chain, scattered so cross-engine latency
                # hides behind GEMM1 groups instead of stalling in-order PE
                if c + 1 < NCHUNK:
                    if ht == 10:
                        emit_gate_logits(c + 1)
                    elif ht == 18:
                        emit_gate_bcast(c + 1)
                    elif ht == 26:
                        emit_srec_bcast(c + 1)

            for ot in range(OT):
                w2t = w2p.tile([128, RH], mm_dt, tag="w2t")
                dma = nc.sync.dma_start(w2t[:], w2[ot, :, :])
                if c == 0:
                    tile.add_dep_helper(dma.ins, w1_last[0],
                                        reason="w2 stream after w1 bulk load")
                ps2 = p2.tile([128, CHUNK], f32, tag="ps2")
                for k2 in range(K2T):
                    nc.tensor.matmul(ps2[:],
                                     w2t[:, k2 * 128:(k2 + 1) * 128],
                                     hgs[k2][:],
                                     start=(k2 == 0), stop=False)
                nc.tensor.matmul(ps2[:], b2_sb[:, ot * 128:(ot + 1) * 128],
                                 gate_bf[:, sl], start=False, stop=True)
                osb = outp.tile([128, CHUNK], f32, tag="osb")
                nc.vector.tensor_mul(osb[:], ps2[:], srecs[c][:])
                nc.sync.dma_start(outT[ot, :, sl], osb[:])
            del gbcs[c], srecs[c], grows[c], recs[c]

    nc.compile()
    return nc


def _get_nc(mm_dt_name="bfloat16"):
    if mm_dt_name not in _NC_CACHE:
        _NC_CACHE[mm_dt_name] = _build_nc(mm_dt_name)
    return _NC_CACHE[mm_dt_name]


def _prepare_in_maps(inputs, np_mm_dtype):
    x = np.asarray(inputs["x"], np.float32)
    in_proj_w = np.asarray(inputs["in_proj_w"], np.float32)
    in_proj_b = np.asarray(inputs["in_proj_b"], np.float32)
    out_proj_w = np.asarray(inputs["out_proj_w"], np.float32)
    out_proj_b = np.asarray(inputs["out_proj_b"], np.float32)
    W1 = np.asarray(inputs["W1"], np.float32)
    b1 = np.asarray(inputs["b1"], np.float32)
    W2 = np.asarray(inputs["W2"], np.float32)
    b2 = np.asarray(inputs["b2"], np.float32)
    Wg = np.asarray(inputs["Wg"], np.float32)
    bg = np.asarray(inputs["bg"], np.float32)

    Wv = in_proj_w[2 * D:]
    bv = in_proj_b[2 * D:]
    A = out_proj_w @ Wv                       # [D, D]
    ba = out_proj_w @ bv + out_proj_b         # [D]
    W1r = W1.reshape(RH, D)
    W1f = W1r @ A                             # [RH, D]
    b1f = W1r @ ba + b1.reshape(RH)           # [RH]
    W2cat = W2.transpose(0, 2, 1).reshape(RH, DOUT)

    w1t_np = np.ascontiguousarray(W1f.T).reshape(KT, 128, RH)
    b1v_np = np.ascontiguousarray(b1f.reshape(HT, 128).T)
    w2_np = np.ascontiguousarray(
        W2cat.reshape(K2T, 128, OT, 128).transpose(2, 1, 0, 3)
    ).reshape(OT, 128, RH)
    wgt_np = np.ascontiguousarray(Wg.T).reshape(KT, 128, R)
    bg_np = np.ascontiguousarray(bg.reshape(R, 1))

    shared = {
        "w1t": w1t_np.astype(np_mm_dtype),
        "b1v": b1v_np,
        "w2": w2_np.astype(np_mm_dtype),
        "b2d": b2.astype(np_mm_dtype),
        "wgt": wgt_np.astype(np_mm_dtype),
        "bgd": bg_np,
    }
    in_maps = []
    for c in range(NCORES):
        xs = x[c * BS:(c + 1) * BS]           # [BS, D]
        xT_np = np.ascontiguousarray(xs.T).reshape(KT, 128, BS)
        m = dict(shared)
        m["xT"] = xT_np.astype(np_mm_dtype)
        in_maps.append(m)
    return in_maps


def _run(inputs, trace=False, mm_dt_name="bfloat16"):
    import ml_dtypes
    from concourse.bass_utils import run_bass_kernel_spmd

    np_mm = ml_dtypes.bfloat16 if mm_dt_name == "bfloat16" else np.float32
    nc = _get_nc(mm_dt_name)
    in_maps = _prepare_in_maps(inputs, np_mm)
    res = run_bass_kernel_spmd(nc, in_maps, list(range(NCORES)), trace=trace)
    out = np.empty((B, DOUT), np.float32)
    for c in range(NCORES):
        out[c * BS:(c + 1) * BS] = res.results[c]["outT"].reshape(DOUT, BS).T
    return out, res


def kernel(**inputs):
    out, _ = _run(inputs, trace=False)
    return out


# revision 36
# speedup vs baseline: 1.0420x; 1.0062x over previous
"""Trainium2 Bass kernel for nn_BrainInspiredRouter.

Math (reference, seq_len==1 attention => attn collapses to the V path):
    attended = x @ (out_proj_w @ Wv).T + (out_proj_w @ bv + out_proj_b)
    h        = relu(attended @ W1[r].T + b1[r])          per route r
    route    = h @ W2[r].T + b2[r]
    gate     = softmax(x @ Wg.T + bg)
    out      = sum_r gate[:, r] * route[:, r, :]

Host-side constant folding (weights only, no activations):
    W1f[r]  = W1[r] @ (out_proj_w @ Wv)      -> h = relu(x @ W1f.T + b1f)
    b1f[r]  = W1[r] @ (out_proj_w@bv + out_proj_b) + b1[r]
    W2cat   = W2.transpose(0,2,1).reshape(R*DH, DOUT)
    out     = (gate*h_flat) @ W2cat + gate @ b2

Device (per core, batch-sharded 8 ways, 2048 rows each; all in feature-major
"T" layout so both GEMMs chain without transposes):
    gate phase: logitsT[8,b] -> E=exp(+bg) -> S=1@E -> rec -> gate_bf[8,b]
    main loop per 512-col batch chunk:
      GEMM1: psum[h,b] = sum_k w1tT[k,h-tile] x xT[k,b]   (bf16 MMs)
      evict: ACT relu(+b1f) -> f32 tmp; DVE tmp*gate_bcast -> bf16 Hg
      GEMM2: psum[o,b] = sum_k2 w2[k2,o-tile] x Hg[k2,b] + b2 x gate_bf
      evict: DVE copy -> f32 -> DMA outT
"""

import numpy as np

B, D, DOUT, R = 16384, 1024, 1024, 8
DH = D // 2            # 512
RH = R * DH            # 4096
NCORES = 8
BS = B // NCORES       # 2048 rows per core
CHUNK = 512
NCHUNK = BS // CHUNK   # 4
KT = D // 128          # 8 k-tiles over D
HT = RH // 128         # 32 h-tiles
K2T = RH // 128        # 32 k-tiles over RH
OT = DOUT // 128       # 8 out-tiles
GRP = DH // 128        # 4 h-tiles per route

_NC_CACHE = {}


def _build_nc(mm_dt_name="bfloat16"):
    from contextlib import ExitStack

    import concourse.bass as bass
    import concourse.mybir as mybir
    import concourse.tile as tile
    from concourse import bacc

    mm_dt = getattr(mybir.dt, mm_dt_name)
    f32 = mybir.dt.float32
    AF = mybir.ActivationFunctionType

    nc = bacc.Bacc("TRN2", target_bir_lowering=False, debug=False,
                   num_devices=NCORES)

    xT = nc.dram_tensor("xT", [KT, 128, BS], mm_dt, kind="ExternalInput")
    w1t = nc.dram_tensor("w1t", [KT, 128, RH], mm_dt, kind="ExternalInput")
    b1v = nc.dram_tensor("b1v", [128, HT], f32, kind="ExternalInput")
    w2 = nc.dram_tensor("w2", [OT, 128, RH], mm_dt, kind="ExternalInput")
    b2d = nc.dram_tensor("b2d", [R, DOUT], mm_dt, kind="ExternalInput")
    wgt = nc.dram_tensor("wgt", [128, KT * R], mm_dt, kind="ExternalInput")
    bgd = nc.dram_tensor("bgd", [R, 1], f32, kind="ExternalInput")
    outT = nc.dram_tensor("outT", [OT, 128, BS], f32, kind="ExternalOutput")
    gate_scr = nc.dram_tensor("gate_scr", [R, BS], mm_dt)
    srec_scr = nc.dram_tensor("srec_scr", [1, BS], f32)

    with tile.TileContext(nc) as tc, ExitStack() as ctx:
        const = ctx.enter_context(tc.tile_pool(name="const", bufs=1))

        # small consts first so the gate phase isn't stuck behind bulk DMA
        bg_sb = const.tile([R, 1], f32, tag="bg")
        nc.sync.dma_start(bg_sb[:], bgd[:, :])
        ones8b = const.tile([R, 1], mm_dt, tag="ones8b")
        nc.any.memset(ones8b[:], 1.0)
        gate_bf = const.tile([R, BS], mm_dt, tag="gatebf")  # unnormalized exp(logits)
        # single consolidated load (per-k loads chop into 16B packets)
        wg_all = const.tile([128, KT * R], mm_dt, tag="wg_all")
        nc.sync.dma_start(wg_all[:], wgt[:, :])
        wg_sb = [wg_all[:, k * R:(k + 1) * R] for k in range(KT)]

        xp = ctx.enter_context(tc.tile_pool(name="xp", bufs=2))
        gm = ctx.enter_context(tc.tile_pool(name="gm", bufs=2))
        gbcp = ctx.enter_context(tc.tile_pool(name="gbcp", bufs=2))
        srecp = ctx.enter_context(tc.tile_pool(name="srecp", bufs=2))
        hgp = ctx.enter_context(tc.tile_pool(name="hgp", bufs=1))
        tmpp = ctx.enter_context(tc.tile_pool(name="tmpp", bufs=3))
        w2p = ctx.enter_context(tc.tile_pool(name="w2p", bufs=3))
        outp = ctx.enter_context(tc.tile_pool(name="outp", bufs=3))
        p1 = ctx.enter_context(tc.tile_pool(name="p1", bufs=4, space="PSUM"))
        p2 = ctx.enter_context(tc.tile_pool(name="p2", bufs=2, space="PSUM"))
        pbc = ctx.enter_context(tc.tile_pool(name="pbc", bufs=2, space="PSUM"))

        xtiles = {}
        xdmas = {}
        w1_last = [None]  # last w1 DMA inst; gates early competing DMAs

        def emit_x_prefetch(c):
            sl = slice(c * CHUNK, (c + 1) * CHUNK)
            xtiles[c] = []
            xdmas[c] = []
            for k in range(KT):
                xt = xp.tile([128, CHUNK], mm_dt, tag=f"xt{k}",
                             name=f"xt{k}_{c}")
                dma = nc.sync.dma_start(xt[:], xT[k, :, sl])
                if c == 1 and w1_last[0] is not None:
                    tile.add_dep_helper(dma.ins, w1_last[0],
                                        reason="x(1) after w1 bulk load")
                xtiles[c].append(xt)
                xdmas[c].append(dma.ins)

        gbcs = {}
        grows = {}
        recs = {}
        srecs = {}

        def emit_gate_logits(c):
            """E = exp(x@Wg.T + bg) for chunk c -> gate_bf[:, c] (bf16,
            unnormalized); the 1/sum factor is applied at GEMM2 eviction."""
            sl = slice(c * CHUNK, (c + 1) * CHUNK)
            pg = pbc.tile([R, CHUNK], f32, tag="pb", name=f"pg_{c}")
            for k in range(KT):
                nc.tensor.matmul(pg[:], wg_sb[k][:], xtiles[c][k][:],
                                 start=(k == 0), stop=(k == KT - 1))
            E = gm.tile([R, CHUNK], f32, tag="E", name=f"E_{c}")
            nc.scalar.activation(E[:], pg[:], AF.Exp, bias=bg_sb[:])
            nc.vector.tensor_copy(gate_bf[:, sl], E[:])
            # round-trip through DRAM so DMA can replicate rows across
            # partitions (step-0 source AP); dep edges added explicitly
            grows[c] = nc.sync.dma_start(gate_scr[:, sl], gate_bf[:, sl])

        def emit_gate_bcast_rows(c, rs):
            """E rows -> 128-partition tiles via replicating DMA."""
            gbcs.setdefault(c, [])
            for r in rs:
                g = gbcp.tile([128, CHUNK], mm_dt, tag=f"gbc{r}",
                              name=f"gbc{r}_{c}")
                src = bass.AP(gate_scr, r * BS + c * CHUNK,
                              [[0, 128], [1, CHUNK]])
                dma = nc.sync.dma_start(g[:], src)
                tile.add_dep_helper(dma.ins, grows[c].ins,
                                    reason="gate bcast read after scr write")
                gbcs[c].append(g)

        def emit_gate_sum(c):
            """S = sum_r E -> 1/S."""
            sl = slice(c * CHUNK, (c + 1) * CHUNK)
            ps = pbc.tile([1, CHUNK], f32, tag="pb", name=f"ps_{c}")
            nc.tensor.matmul(ps[:], ones8b[:], gate_bf[:, sl],
                             start=True, stop=True)
            rec = gm.tile([1, CHUNK], f32, tag="rec", name=f"rec_{c}")
            nc.vector.reciprocal(rec[:], ps[:])
            recs[c] = rec

        def emit_gate_bcast(c):
            emit_gate_bcast_rows(c, range(R))
            emit_gate_sum(c)

        def emit_srec_bcast(c):
            """broadcast 1/S to 128 partitions via replicating DMA."""
            sl = slice(c * CHUNK, (c + 1) * CHUNK)
            w = nc.sync.dma_start(srec_scr[:, sl], recs[c][:])
            srec = srecp.tile([128, CHUNK], f32, tag="srec",
                              name=f"srec_{c}")
            src = bass.AP(srec_scr, c * CHUNK, [[0, 128], [1, CHUNK]])
            dma = nc.sync.dma_start(srec[:], src)
            tile.add_dep_helper(dma.ins, w.ins,
                                reason="srec bcast read after scr write")
            srecs[c] = srec

        # prologue: x for chunk 0, gate(0) logits, then the bulk weight
        # loads. The bcast/srec matmuls are interleaved into chunk-0's
        # first GEMM1 groups to fill the w1-DMA-arrival bubbles.
        emit_x_prefetch(0)
        emit_gate_logits(0)

        w1_sb = [const.tile([128, RH], mm_dt, tag=f"w1_{k}", name=f"w1sb{k}")
                 for k in range(KT)]
        # load in h-quarters, all k per quarter, with dep chains so the
        # quarters actually ARRIVE in order (HW queues otherwise serve all
        # transfers round-robin and everything lands together): GEMM1(0)'s
        # first h-tiles unblock after 2MB instead of the full 8MB
        Q = RH // 4
        prev_wave = xdmas[0]
        for q in range(4):
            wave = []
            for k in range(KT):
                dma = nc.sync.dma_start(w1_sb[k][:, q * Q:(q + 1) * Q],
                                        w1t[k, :, q * Q:(q + 1) * Q])
                tile.add_dep_helper(dma.ins, prev_wave[k % len(prev_wave)],
                                    reason=f"w1 q{q} wave order")
                wave.append(dma.ins)
            prev_wave = wave
        w1_last[0] = prev_wave[-1]
        b1_sb = const.tile([128, HT], f32, tag="b1")
        nc.sync.dma_start(b1_sb[:], b1v[:, :])
        b2_sb = const.tile([R, DOUT], mm_dt, tag="b2")
        nc.sync.dma_start(b2_sb[:], b2d[:, :])

        for c in range(NCHUNK):
            sl = slice(c * CHUNK, (c + 1) * CHUNK)
            xts = xtiles.pop(c)
            if c + 1 < NCHUNK:
                emit_x_prefetch(c + 1)

            def evict_h(ht, ps1, hgs):
                tmp = tmpp.tile([128, CHUNK], f32, tag="tmp", name=f"tmp_{c}_{ht}")
                nc.scalar.activation(tmp[:], ps1[:], AF.Relu,
                                     bias=b1_sb[:, ht:ht + 1])
                hg = hgp.tile([128, CHUNK], mm_dt, tag=f"hg{ht}",
                              name=f"hg{ht}_{c}")
                nc.vector.tensor_mul(hg[:], tmp[:], gbcs[c][ht // GRP][:])
                hgs.append(hg)

            hgs = []
            if c == 0:
                # While w1 streams in, run ht groups pairwise with
                # interleaved k-loops (2 MMs ready per w1[k] arrival) and
                # slot the gate-aux matmuls into the remaining bubbles.
                for hp in range(4):
                    ha, hb = 2 * hp, 2 * hp + 1
                    pa = p1.tile([128, CHUNK], f32, tag="ps1", name=f"pa_{hp}")
                    pb2 = p1.tile([128, CHUNK], f32, tag="ps1", name=f"pb2_{hp}")
                    for k in range(KT):
                        nc.tensor.matmul(pa[:],
                                         w1_sb[k][:, ha * 128:(ha + 1) * 128],
                                         xts[k][:],
                                         start=(k == 0), stop=(k == KT - 1))
                        if hp == 0 and k >= 4:
                            emit_gate_bcast_rows(0, [2 * (k - 4)])
                        elif hp == 1 and k == 2:
                            emit_srec_bcast(0)
                        nc.tensor.matmul(pb2[:],
                                         w1_sb[k][:, hb * 128:(hb + 1) * 128],
                                         xts[k][:],
                                         start=(k == 0), stop=(k == KT - 1))
                        if hp == 0 and k >= 4:
                            emit_gate_bcast_rows(0, [2 * (k - 4) + 1])
                    if hp == 0:
                        emit_gate_sum(0)
                    evict_h(ha, pa, hgs)
                    evict_h(hb, pb2, hgs)
                ht_start = 8
            else:
                ht_start = 0

            for ht in range(ht_start, HT):
                ps1 = p1.tile([128, CHUNK], f32, tag="ps1")
                for k in range(KT):
                    nc.tensor.matmul(ps1[:],
                                     w1_sb[k][:, ht * 128:(ht + 1) * 128],
                                     xts[k][:],
                                     start=(k == 0), stop=(k == KT - 1))
                evict_h(ht, ps1, hgs)
                # next chunk's gate chain, scattered so cross-engine latency
                # hides behind GEMM1 groups instead of stalling in-order PE
                if c + 1 < NCHUNK:
                    if ht == 10:
                        emit_gate_logits(c + 1)
                    elif ht == 18:
                        emit_gate_bcast(c + 1)
                    elif ht == 26:
                        emit_srec_bcast(c + 1)

            for ot in range(OT):
                w2t = w2p.tile([128, RH], mm_dt, tag="w2t")
                dma = nc.sync.dma_start(w2t[:], w2[ot, :, :])
                if c == 0:
                    tile.add_dep_helper(dma.ins, w1_last[0],
                                        reason="w2 stream after w1 bulk load")
                ps2 = p2.tile([128, CHUNK], f32, tag="ps2")
                for k2 in range(K2T):
                    nc.tensor.matmul(ps2[:],
                                     w2t[:, k2 * 128:(k2 + 1) * 128],
                                     hgs[k2][:],
                                     start=(k2 == 0), stop=False)
                nc.tensor.matmul(ps2[:], b2_sb[:, ot * 128:(ot + 1) * 128],
                                 gate_bf[:, sl], start=False, stop=True)
                osb = outp.tile([128, CHUNK], f32, tag="osb")
                nc.vector.tensor_mul(osb[:], ps2[:], srecs[c][:])
                nc.sync.dma_start(outT[ot, :, sl], osb[:])
            del gbcs[c], srecs[c], grows[c], recs[c]

    nc.compile()
    return nc


def _get_nc(mm_dt_name="bfloat16"):
    if mm_dt_name not in _NC_CACHE:
        _NC_CACHE[mm_dt_name] = _build_nc(mm_dt_name)
    return _NC_CACHE[mm_dt_name]


def _prepare_in_maps(inputs, np_mm_dtype):
    x = np.asarray(inputs["x"], np.float32)
    in_proj_w = np.asarray(inputs["in_proj_w"], np.float32)
    in_proj_b = np.asarray(inputs["in_proj_b"], np.float32)
    out_proj_w = np.asarray(inputs["out_proj_w"], np.float32)
    out_proj_b = np.asarray(inputs["out_proj_b"], np.float32)
    W1 = np.asarray(inputs["W1"], np.float32)
    b1 = np.asarray(inputs["b1"], np.float32)
    W2 = np.asarray(inputs["W2"], np.float32)
    b2 = np.asarray(inputs["b2"], np.float32)
    Wg = np.asarray(inputs["Wg"], np.float32)
    bg = np.asarray(inputs["bg"], np.float32)

    Wv = in_proj_w[2 * D:]
    bv = in_proj_b[2 * D:]
    A = out_proj_w @ Wv                       # [D, D]
    ba = out_proj_w @ bv + out_proj_b         # [D]
    W1r = W1.reshape(RH, D)
    W1f = W1r @ A                             # [RH, D]
    b1f = W1r @ ba + b1.reshape(RH)           # [RH]
    W2cat = W2.transpose(0, 2, 1).reshape(RH, DOUT)

    w1t_np = np.ascontiguousarray(W1f.T).reshape(KT, 128, RH)
    b1v_np = np.ascontiguousarray(b1f.reshape(HT, 128).T)
    w2_np = np.ascontiguousarray(
        W2cat.reshape(K2T, 128, OT, 128).transpose(2, 1, 0, 3)
    ).reshape(OT, 128, RH)
    # [p, k*R+r] = Wg[r, k*128+p]: 128B-contiguous per partition line
    wgt_np = np.ascontiguousarray(Wg.reshape(R, KT, 128).transpose(2, 1, 0)
                                  ).reshape(128, KT * R)
    bg_np = np.ascontiguousarray(bg.reshape(R, 1))

    shared = {
        "w1t": w1t_np.astype(np_mm_dtype),
        "b1v": b1v_np,
        "w2": w2_np.astype(np_mm_dtype),
        "b2d": b2.astype(np_mm_dtype),
        "wgt": wgt_np.astype(np_mm_dtype),
        "bgd": bg_np,
    }
    in_maps = []
    for c in range(NCORES):
        xs = x[c * BS:(c + 1) * BS]           # [BS, D]
        xT_np = np.ascontiguousarray(xs.T).reshape(KT, 128, BS)
        m = dict(shared)
        m["xT"] = xT_np.astype(np_mm_dtype)
        in_maps.append(m)
    return in_maps


def _run(inputs, trace=False, mm_dt_name="bfloat16"):
    import ml_dtypes
    from concourse.bass_utils import run_bass_kernel_spmd

    np_mm = ml_dtypes.bfloat16 if mm_dt_name == "bfloat16" else np.float32
    nc = _get_nc(mm_dt_name)
    in_maps = _prepare_in_maps(inputs, np_mm)
    res = run_bass_kernel_spmd(nc, in_maps, list(range(NCORES)), trace=trace)
    out = np.empty((B, DOUT), np.float32)
    for c in range(NCORES):
        out[c * BS:(c + 1) * BS] = res.results[c]["outT"].reshape(DOUT, BS).T
    return out, res


def kernel(**inputs):
    out, _ = _run(inputs, trace=False)
    return out
